# revision 31
# baseline (speedup 1.0000x reference)
"""Multi-head attention (bsz=2, seq=2048, hidden=1024, heads=16) on 8 TRN2 cores.

Sharding: core c = 4*b + g handles batch b and heads [4g, 4g+4).
Each core computes Q/K/V projections for its 4 heads, causal softmax
attention, and a partial output projection over its 256 features; the host
sums the 4 per-batch partials.

All matmuls run in bfloat16 (1 cycle/row streaming); PSUM accumulation
stays fp32, so the only precision loss is the bf16 quantization of
inputs/intermediates (~4e-3 l2 rel err vs the fp32 reference).

Scores are computed transposed (S^T: k on partitions, q on free dim) so the
probabilities feed P@V directly as the moving operand. V is augmented with
64 all-ones columns so the PV matmul also produces the softmax denominator
replicated across 64 partitions. Scores stay in +-2.5 so exp() needs no
max-subtraction; causal masking is a 0/1 multiply on the probabilities.

Scheduling is built around keeping the PE queue dense (the HAM clock gate
halves the PE clock for ~3us after any idle gap): per 512-query chunk the
4 heads' score/PV chains run round-robin with PV lagging one k-block, and
projection groups (this chunk's V, the next chunk's Q/K, the previous
chunk's O) are interleaved between attention rounds as filler so the PE
never waits on the ACT exp chain. Inputs arrive as a few large packed DMAs
(4KB/partition descriptors) split across the SP and ACT hardware DGE
queues; outputs leave as one packed store per 128-query block.
"""

import sys

sys.path.insert(0, "/opt/trn_rl_repo")

from contextlib import ExitStack

import ml_dtypes
import numpy as np

import concourse.tile as tile
from concourse import bacc, bass_utils, mybir

B, S, H = 2, 2048, 1024
NHC = 4  # heads per core
HD = 64  # head dim
F = NHC * HD  # features per core (256)
N_CORES = 8
QC = 512  # query-chunk width
KB = 128  # key-block size
SCALE = 1.0 / 8.0  # 1/sqrt(HD)

F32 = mybir.dt.float32
BF16 = mybir.dt.bfloat16
EXP = mybir.ActivationFunctionType.Exp
COPY = mybir.ActivationFunctionType.Copy

_CACHE = {}


def _emit(tc):
    nc = tc.nc
    # packed layouts: per-partition segments are 4KB so DMA descriptors are
    # large; one dma_start per [128, 2048] tile
    xP_d = nc.dram_tensor("xP", [4, 2, KB, 4 * QC], BF16, kind="ExternalInput").ap()
    wqP_d = nc.dram_tensor("wqP", [2, KB, 8 * KB], BF16, kind="ExternalInput").ap()
    wkP_d = nc.dram_tensor("wkP", [2, KB, 8 * KB], BF16, kind="ExternalInput").ap()
    wvP_d = nc.dram_tensor("wvP", [2, KB, 4 * F], BF16, kind="ExternalInput").ap()
    woP_d = nc.dram_tensor("woP", [KB, 2 * H], BF16, kind="ExternalInput").ap()
    mtri_d = nc.dram_tensor("mtri", [KB, KB], BF16, kind="ExternalInput").ap()
    out_d = nc.dram_tensor("out", [S, H], BF16, kind="ExternalOutput").ap()

    ctx = tc._emit_ctx
    const = ctx.enter_context(tc.tile_pool(name="const", bufs=1))
    persist = ctx.enter_context(tc.tile_pool(name="persist", bufs=1))
    xpool = ctx.enter_context(tc.tile_pool(name="xc", bufs=4))
    pt_pool = ctx.enter_context(tc.tile_pool(name="pt", bufs=8))
    recip_pool = ctx.enter_context(tc.tile_pool(name="recip", bufs=8))
    ostage = ctx.enter_context(tc.tile_pool(name="ostage", bufs=2))
    # two 2-bank slots: score supertiles for a head-pair, also sliced by the
    # projection groups; plus 4 single-bank per-head PV accumulators
    ps_st = ctx.enter_context(tc.tile_pool(name="psst", bufs=2, space="PSUM"))
    ps_po = ctx.enter_context(tc.tile_pool(name="pspo", bufs=4, space="PSUM"))

    # wq/wk are packed per output-feature half (fc): wqf[fc][:, hc*128...]
    # is the [128, 128] stationary for hidden block hc
    wqf = [persist.tile([KB, 8 * KB], BF16, tag=f"wq{f}", name=f"wq{f}") for f in range(2)]
    wkf = [persist.tile([KB, 8 * KB], BF16, tag=f"wk{f}", name=f"wk{f}") for f in range(2)]
    wvt = [persist.tile([KB, 4 * F], BF16, tag=f"wv{i}", name=f"wv{i}") for i in range(2)]
    wot = persist.tile([KB, 2 * H], BF16, tag="wo", name="wo")
    wv = [wvt[i // 4][:, (i % 4) * F : (i % 4 + 1) * F] for i in range(8)]
    wo = [wot[:, i * H : (i + 1) * H] for i in range(2)]

    qts = [persist.tile([KB, S], BF16, tag=f"qt{i}", name=f"qt{i}") for i in range(2)]
    kts = [persist.tile([KB, S], BF16, tag=f"kt{i}", name=f"kt{i}") for i in range(2)]
    # V_aug per k-block: [V_h (64) | ones (64)] per head -> [128, 512]
    vts = [
        persist.tile([KB, NHC * 2 * HD], BF16, tag=f"vt{i}", name=f"vt{i}")
        for i in range(16)
    ]
    ats = [persist.tile([KB, S], BF16, tag=f"at{i}", name=f"at{i}") for i in range(2)]

    xp = [[None, None] for _ in range(4)]  # xp[jq][half] = [128, 2048]

    def load_x(jq, eng):
        for half in range(2):
            t = xpool.tile([KB, 4 * QC], BF16, tag="xc", name=f"xp{jq}_{half}")
            eng.dma_start(t[:], xP_d[jq, half])
            xp[jq][half] = t

    def xcs(jq, hc):  # [128, 512] view of x chunk jq, hidden block hc
        return xp[jq][hc // 4][:, (hc % 4) * QC : (hc % 4 + 1) * QC]

    mtri = const.tile([KB, KB], BF16, tag="mtri")
    ones_bf = const.tile([KB, NHC * HD], BF16, tag="ones16")

    # SP: wq/wk fc-halves in consumption order; ACT: x chunks 0/1, mtri;
    # gpsimd (SWDGE): wv halves + wo, a third parallel trigger path
    nc.sync.dma_start(wqf[0][:], wqP_d[0])
    load_x(0, nc.scalar)
    nc.sync.dma_start(wkf[0][:], wkP_d[0])
    nc.sync.dma_start(wvt[0][:], wvP_d[0])
    nc.sync.dma_start(wqf[1][:], wqP_d[1])
    nc.sync.dma_start(wkf[1][:], wkP_d[1])
    nc.scalar.dma_start(mtri[:], mtri_d[:])
    nc.vector.memset(ones_bf[:], 1.0)
    load_x(1, nc.scalar)
    nc.sync.dma_start(wvt[1][:], wvP_d[1])
    nc.sync.dma_start(wot[:], woP_d[:])

    # ---- filler emitters: one call emits one projection group ----
    # all ps_st allocations are full [KB, 2*QC] slots so the pool slots are
    # uniform; projection groups just use a slice
    def qk_group(jq, wf, dst, fc):
        ps = ps_st.tile([KB, 2 * QC], F32, tag="st", name=f"pp{jq}_{fc}")[:, 0:QC]
        for hc in range(8):
            nc.tensor.matmul(
                ps[:],
                wf[fc][:, hc * KB : (hc + 1) * KB],
                xcs(jq, hc),
                start=(hc == 0),
                stop=(hc == 7),
            )
        nc.vector.tensor_copy(dst[fc][:, jq * QC : (jq + 1) * QC], ps[:])

    def v_group(jq, sub):
        rc = 4 * jq + sub
        psv = ps_st.tile([KB, 2 * QC], F32, tag="st", name=f"pv{rc}")[:, 0:F]
        for hc in range(8):
            nc.tensor.matmul(
                psv[:],
                xcs(jq, hc)[:, sub * KB : (sub + 1) * KB],
                wv[hc][:],
                start=(hc == 0),
                stop=(hc == 7),
            )
        v_heads = vts[rc][:].rearrange("p (h d) -> p h d", h=NHC)
        nc.vector.tensor_copy(
            v_heads[:, :, 0:HD], psv[:].rearrange("p (h d) -> p h d", h=NHC)
        )
        nc.vector.tensor_copy(
            v_heads[:, :, HD : 2 * HD],
            ones_bf[:, :].rearrange("p (h d) -> p h d", h=NHC),
        )

    def o_group(qb, act_copy=False):
        # pso lives in the st pool: during attention rounds the po pool's 4
        # banks are all held by the per-head accumulators, so allocating pso
        # there would deadlock the in-order PE queue.
        ost = ostage.tile([KB, 2 * QC], BF16, tag="ost", name=f"os{qb}")
        for oc in range(2):
            pso = ps_st.tile([KB, 2 * QC], F32, tag="st", name=f"pso{qb}_{oc}")
            pso = pso[:, 0:QC]
            for fc in range(2):
                nc.tensor.matmul(
                    pso[:],
                    ats[fc][:, qb * KB : (qb + 1) * KB],
                    wo[fc][:, oc * QC : (oc + 1) * QC],
                    start=(fc == 0),
                    stop=(fc == 1),
                )
            dst = ost[:, oc * QC : (oc + 1) * QC]
            if act_copy and oc == 0:
                nc.scalar.activation(dst, pso[:], COPY)
            else:
                nc.vector.tensor_copy(dst, pso[:])
        nc.sync.dma_start(out_d[qb * KB : (qb + 1) * KB, :], ost[:])

    # chunk 0's projections run before its attention
    for fc in range(2):
        qk_group(0, wqf, qts, fc)
        qk_group(0, wkf, kts, fc)
    for sub in range(4):
        v_group(0, sub)

    for jq in range(4):
        q0 = jq * QC
        nkb = 4 * jq + 4

        # filler work interleaved into this chunk's attention rounds, in
        # dependency order: this chunk's remaining V groups (needed by the
        # last 4 rounds only), next chunk's x prefetch + Q/K projections,
        # then the previous chunk's O projection (ats ready ~round 1).
        fillers = []
        if jq > 0:
            for sub in range(4):
                fillers.append(lambda j=jq, s=sub: v_group(j, s))
        if jq + 2 < 4:
            fillers.append(lambda j=jq + 2: load_x(j, nc.sync))
        if jq + 1 < 4:
            for fc in range(2):
                fillers.append(lambda j=jq + 1, f=fc: qk_group(j, wqf, qts, f))
                fillers.append(lambda j=jq + 1, f=fc: qk_group(j, wkf, kts, f))
        if jq > 0:
            for sub in range(4):
                fillers.append(lambda q=4 * (jq - 1) + sub: o_group(q))
        # spread fillers evenly over rounds 1..nkb-1 (round 0's PSUM slots
        # feed the next rounds' scores; a filler there funnels the PE queue
        # behind the first exp)
        # the last two filler groups are held back and emitted after the
        # final PV flush: they keep the PE warm through the normalize
        # chain at the chunk boundary
        reserved = fillers[-2:] if len(fillers) >= 2 else []
        fillers = fillers[: len(fillers) - len(reserved)]
        nf = len(fillers)
        sched = [min(nkb - 1, 1 + (i * (nkb - 1)) // nf) for i in range(nf)]
        fi = 0

        # ---- attention: 4 heads round-robin, PV lagging one k-block.
        # Scores for a head-pair land in one 2-bank PSUM supertile so a
        # single exp covers both heads (halves ACT per-instruction
        # overhead, the round pacer). ----
        po = [
            ps_po.tile([KB, QC], F32, tag="ot", name=f"po{jq}_{h}") for h in range(NHC)
        ]
        pend = []  # (pts, w0, ik) awaiting their PV pass; lag 2 rounds

        def flush_pv():
            pts_, w0_, ik_ = pend.pop(0)
            for h in range(NHC):
                pt2, base = pts_[h]
                nc.tensor.matmul(
                    po[h][:, w0_:QC],
                    vts[ik_][:, h * 2 * HD : (h + 1) * 2 * HD],
                    pt2[:, base + w0_ : base + QC],
                    start=(ik_ == 0),
                    stop=(ik_ == nkb - 1),
                )

        for ik in range(nkb):
            r = ik - 4 * jq
            w0 = max(r, 0) * KB  # fully-masked leading columns skipped
            pts = []
            for p in range(2):  # head pair (2p, 2p+1)
                st2 = ps_st.tile([KB, 2 * QC], F32, tag="st", name=f"st{jq}_{p}_{ik}")
                pt2 = pt_pool.tile([KB, 2 * QC], BF16, tag="pt", name=f"pt{jq}_{p}_{ik}")
                for hh in range(2):
                    h = 2 * p + hh
                    t, po_ = h // 2, (h % 2) * HD
                    nc.tensor.matmul(
                        st2[:, hh * QC + w0 : (hh + 1) * QC],
                        kts[t][po_ : po_ + HD, ik * KB : (ik + 1) * KB],
                        qts[t][po_ : po_ + HD, q0 + w0 : q0 + QC],
                        start=True,
                        stop=True,
                    )
                sv = st2[:].rearrange("p (h q) -> p h q", h=2)[:, :, w0:QC]
                pv_ = pt2[:].rearrange("p (h q) -> p h q", h=2)[:, :, w0:QC]
                nc.scalar.activation(pv_, sv, EXP, scale=SCALE)
                if r >= 0:  # causal triangle mask on the diagonal block
                    for hh in range(2):
                        tri = pt2[:, hh * QC + r * KB : hh * QC + (r + 1) * KB]
                        nc.vector.tensor_mul(tri, tri, mtri[:])
                pts.extend((pt2, hh * QC) for hh in range(2))
            pend.append((pts, w0, ik))
            if len(pend) > 3:
                flush_pv()
            while fi < nf and sched[fi] <= ik:
                fillers[fi]()
                fi += 1
        while fi < nf:
            fillers[fi]()
            fi += 1
        while pend:
            flush_pv()
        # per-head normalize: stage the denominator (rows 64:128 of po, 64
        # replicated copies) to SBUF, approx-reciprocal, scale.  All on DVE
        # as per-head chains so the ACT queue stays free for the next
        # chunk's exps; the PV lag covers the latency.
        for h in range(NHC):
            t, po_ = h // 2, (h % 2) * HD
            den = recip_pool.tile([HD, QC], F32, tag="recip", name=f"dn{jq}_{h}")
            recip = recip_pool.tile([HD, QC], F32, tag="recip", name=f"rc{jq}_{h}")
            if jq == 3:  # no more exps: ACT is idle, keep the DVE chain short
                nc.scalar.activation(den[:], po[h][HD : 2 * HD, :], COPY)
            else:
                nc.vector.tensor_copy(den[:], po[h][HD : 2 * HD, :])
            with nc.allow_low_precision(reason="softmax denom"):
                nc.vector.reciprocal_approx_fast(recip[:], den[:])
                nc.vector.tensor_mul(
                    ats[t][po_ : po_ + HD, q0 : q0 + QC], po[h][0:HD, :], recip[:]
                )
        for fn in reserved:
            fn()

    # last chunk's output projection (ACT is idle here, so it takes half
    # the PSUM->SBUF copies off the DVE critical path)
    for sub in range(4):
        o_group(12 + sub, act_copy=True)



def _build():
    if "nc" in _CACHE:
        return _CACHE["nc"]
    nc = bacc.Bacc(
        "TRN2", target_bir_lowering=False, debug=False, num_devices=N_CORES
    )
    with tile.TileContext(nc) as tc:
        with ExitStack() as ctx:
            tc._emit_ctx = ctx
            _emit(tc)
    nc.compile()
    _CACHE["nc"] = nc
    return nc


def _numpy_fallback(q, attention_mask, Wq, Wk, Wv, Wo):
    import math

    b, s, _ = q.shape
    causal = np.tril(np.ones((s, s), bool))
    valid = attention_mask != 0
    mask = causal[None] & valid[:, :, None] & valid[:, None, :]
    mask = mask[:, None]
    out = np.zeros((b, s, H), np.float32)
    for bi in range(b):
        x = q[bi]
        nh = x.shape[1] // HD
        qh = (x @ Wq.T).reshape(s, nh, HD).transpose(1, 0, 2)
        kh = (x @ Wk.T).reshape(s, nh, HD).transpose(1, 0, 2)
        vh = (x @ Wv.T).reshape(s, nh, HD).transpose(1, 0, 2)
        sc = np.einsum("hqd,hkd->hqk", qh, kh) / math.sqrt(HD)
        sc = np.where(mask[bi], sc, np.float32(-1e6))
        sc = sc - sc.max(-1, keepdims=True)
        e = np.exp(sc)
        p = e / e.sum(-1, keepdims=True)
        p = np.where(mask[bi], p, np.float32(0.0))
        o = np.einsum("hqk,hkd->hqd", p, vh).transpose(1, 0, 2).reshape(s, -1)
        out[bi] = o @ Wo.T
    return out


def _pack_x(xT):
    # xP[jq, half] = [128, 2048]: 4 hidden blocks of x^T side by side
    bf = ml_dtypes.bfloat16
    xP = np.empty((4, 2, KB, 4 * QC), dtype=bf)
    for jq in range(4):
        for half in range(2):
            for i in range(4):
                hc = half * 4 + i
                xP[jq, half, :, i * QC : (i + 1) * QC] = xT[
                    hc * KB : (hc + 1) * KB, jq * QC : (jq + 1) * QC
                ]
    return xP


def _pack_w(wT):
    # [n*128, W] -> [128, n*W]: hidden blocks side by side
    bf = ml_dtypes.bfloat16
    n = wT.shape[0] // KB
    out = np.empty((KB, n * wT.shape[1]), dtype=bf)
    for i in range(n):
        out[:, i * wT.shape[1] : (i + 1) * wT.shape[1]] = wT[i * KB : (i + 1) * KB, :]
    return out


def _pack_w_fc(wT):
    # [1024, 256] -> [2, 128, 1024]: per output-feature half fc, the 8
    # hidden blocks' [128, 128] stationaries side by side
    bf = ml_dtypes.bfloat16
    out = np.empty((2, KB, 8 * KB), dtype=bf)
    for fc in range(2):
        for hc in range(8):
            out[fc, :, hc * KB : (hc + 1) * KB] = wT[
                hc * KB : (hc + 1) * KB, fc * KB : (fc + 1) * KB
            ]
    return out


def _pack_w_half(wT):
    # [1024, 256] -> [2, 128, 1024]: two halves of 4 hidden blocks each
    bf = ml_dtypes.bfloat16
    out = np.empty((2, KB, 4 * F), dtype=bf)
    for i in range(2):
        for j in range(4):
            out[i, :, j * F : (j + 1) * F] = wT[(4 * i + j) * KB : (4 * i + j + 1) * KB, :]
    return out


def _run(q, attention_mask, Wq, Wk, Wv, Wo, trace=False, **trace_kwargs):
    q = np.ascontiguousarray(np.asarray(q, dtype=np.float32))
    Wq = np.asarray(Wq, dtype=np.float32)
    Wk = np.asarray(Wk, dtype=np.float32)
    Wv = np.asarray(Wv, dtype=np.float32)
    Wo = np.asarray(Wo, dtype=np.float32)
    am = np.asarray(attention_mask)
    if q.shape != (B, S, H) or not np.all(am != 0):
        return _numpy_fallback(q, am, Wq, Wk, Wv, Wo), None

    bf = ml_dtypes.bfloat16
    idx = np.arange(KB)
    mtri = (idx[:, None] <= idx[None, :]).astype(bf)

    in_maps = []
    for c in range(N_CORES):
        b, g = c // 4, c % 4
        fs = slice(F * g, F * (g + 1))
        in_maps.append(
            {
                "xP": _pack_x(q[b].T.astype(bf)),
                "wqP": _pack_w_fc(Wq[fs, :].T.astype(bf)),
                "wkP": _pack_w_fc(Wk[fs, :].T.astype(bf)),
                "wvP": _pack_w_half(Wv[fs, :].T.astype(bf)),
                "woP": _pack_w(Wo[:, fs].T.astype(bf)),
                "mtri": mtri,
            }
        )

    nc = _build()
    res = bass_utils.run_bass_kernel_spmd(
        nc, in_maps, core_ids=list(range(N_CORES)), trace=trace, **trace_kwargs
    )
    outs = [r["out"].astype(np.float32) for r in res.results]
    full = np.empty((B, S, H), np.float32)
    for b in range(B):
        full[b] = outs[4 * b] + outs[4 * b + 1] + outs[4 * b + 2] + outs[4 * b + 3]
    return full, res


def kernel(q, attention_mask, Wq, Wk, Wv, Wo):
    out, _ = _run(q, attention_mask, Wq, Wk, Wv, Wo)
    return out


# revision 33
# speedup vs baseline: 1.1168x; 1.1168x over previous
"""Multi-head attention (bsz=2, seq=2048, hidden=1024, heads=16) on 8 TRN2 cores.

Sharding: core c = 4*b + g handles batch b and heads [4g, 4g+4).
Each core computes Q/K/V projections for its 4 heads, causal softmax
attention, and a partial output projection over its 256 features; the host
sums the 4 per-batch partials.

All matmuls run in bfloat16 (1 cycle/row streaming); PSUM accumulation
stays fp32, so the only precision loss is the bf16 quantization of
inputs/intermediates (~4e-3 l2 rel err vs the fp32 reference).

Scores are computed transposed (S^T: k on partitions, q on free dim) so the
probabilities feed P@V directly as the moving operand. V is augmented with
64 all-ones columns so the PV matmul also produces the softmax denominator
replicated across 64 partitions. Scores stay in +-2.5 so exp() needs no
max-subtraction; causal masking is a 0/1 multiply on the probabilities.

Scheduling is built around keeping the PE queue dense (the HAM clock gate
halves the PE clock for ~3us after any idle gap): per 512-query chunk the
4 heads' score/PV chains run round-robin with PV lagging one k-block, and
projection groups (this chunk's V, the next chunk's Q/K, the previous
chunk's O) are interleaved between attention rounds as filler so the PE
never waits on the ACT exp chain. Inputs arrive as a few large packed DMAs
(4KB/partition descriptors) split across the SP and ACT hardware DGE
queues; outputs leave as one packed store per 128-query block.
"""

import sys

sys.path.insert(0, "/opt/trn_rl_repo")

from contextlib import ExitStack

import ml_dtypes
import numpy as np

import concourse.tile as tile
from concourse import bacc, bass_utils, mybir

B, S, H = 2, 2048, 1024
NHC = 4  # heads per core
HD = 64  # head dim
F = NHC * HD  # features per core (256)
N_CORES = 8
QC = 512  # query-chunk width
KB = 128  # key-block size
SCALE = 1.0 / 8.0  # 1/sqrt(HD)

F32 = mybir.dt.float32
BF16 = mybir.dt.bfloat16
EXP = mybir.ActivationFunctionType.Exp
COPY = mybir.ActivationFunctionType.Copy

_CACHE = {}


def _emit(tc):
    nc = tc.nc
    # packed layouts: per-partition segments are 4KB so DMA descriptors are
    # large; one dma_start per [128, 2048] tile
    xP_d = nc.dram_tensor("xP", [4, 2, KB, 4 * QC], BF16, kind="ExternalInput").ap()
    wqP_d = nc.dram_tensor("wqP", [2, KB, 8 * KB], BF16, kind="ExternalInput").ap()
    wkP_d = nc.dram_tensor("wkP", [2, KB, 8 * KB], BF16, kind="ExternalInput").ap()
    wvP_d = nc.dram_tensor("wvP", [2, KB, 4 * F], BF16, kind="ExternalInput").ap()
    woP_d = nc.dram_tensor("woP", [KB, 2 * H], BF16, kind="ExternalInput").ap()
    mtri_d = nc.dram_tensor("mtri", [KB, KB], BF16, kind="ExternalInput").ap()
    out_d = nc.dram_tensor("out", [S, H], BF16, kind="ExternalOutput").ap()

    ctx = tc._emit_ctx
    const = ctx.enter_context(tc.tile_pool(name="const", bufs=1))
    persist = ctx.enter_context(tc.tile_pool(name="persist", bufs=1))
    xpool = ctx.enter_context(tc.tile_pool(name="xc", bufs=4))
    pt_pool = ctx.enter_context(tc.tile_pool(name="pt", bufs=8))
    recip_pool = ctx.enter_context(tc.tile_pool(name="recip", bufs=8))
    ostage = ctx.enter_context(tc.tile_pool(name="ostage", bufs=2))
    # two 2-bank slots: score supertiles for a head-pair, also sliced by the
    # projection groups; plus 4 single-bank per-head PV accumulators
    ps_st = ctx.enter_context(tc.tile_pool(name="psst", bufs=2, space="PSUM"))
    ps_po = ctx.enter_context(tc.tile_pool(name="pspo", bufs=4, space="PSUM"))

    # wq/wk are packed per output-feature half (fc): wqf[fc][:, hc*128...]
    # is the [128, 128] stationary for hidden block hc
    wqf = [persist.tile([KB, 8 * KB], BF16, tag=f"wq{f}", name=f"wq{f}") for f in range(2)]
    wkf = [persist.tile([KB, 8 * KB], BF16, tag=f"wk{f}", name=f"wk{f}") for f in range(2)]
    wvt = [persist.tile([KB, 4 * F], BF16, tag=f"wv{i}", name=f"wv{i}") for i in range(2)]
    wot = persist.tile([KB, 2 * H], BF16, tag="wo", name="wo")
    wv = [wvt[i // 4][:, (i % 4) * F : (i % 4 + 1) * F] for i in range(8)]
    wo = [wot[:, i * H : (i + 1) * H] for i in range(2)]

    qts = [persist.tile([KB, S], BF16, tag=f"qt{i}", name=f"qt{i}") for i in range(2)]
    kts = [persist.tile([KB, S], BF16, tag=f"kt{i}", name=f"kt{i}") for i in range(2)]
    # V_aug per k-block: [V_h (64) | ones (64)] per head -> [128, 512]
    vts = [
        persist.tile([KB, NHC * 2 * HD], BF16, tag=f"vt{i}", name=f"vt{i}")
        for i in range(16)
    ]
    ats = [persist.tile([KB, S], BF16, tag=f"at{i}", name=f"at{i}") for i in range(2)]

    xp = [[None, None] for _ in range(4)]  # xp[jq][half] = [128, 2048]

    def load_x(jq, eng):
        for half in range(2):
            t = xpool.tile([KB, 4 * QC], BF16, tag="xc", name=f"xp{jq}_{half}")
            eng.dma_start(t[:], xP_d[jq, half])
            xp[jq][half] = t

    def xcs(jq, hc):  # [128, 512] view of x chunk jq, hidden block hc
        return xp[jq][hc // 4][:, (hc % 4) * QC : (hc % 4 + 1) * QC]

    mtri = const.tile([KB, KB], BF16, tag="mtri")
    ones_bf = const.tile([KB, NHC * HD], BF16, tag="ones16")

    # SP: wq/wk fc-halves in consumption order; ACT: x chunks 0/1, mtri;
    # gpsimd (SWDGE): wv halves + wo, a third parallel trigger path
    nc.sync.dma_start(wqf[0][:], wqP_d[0])
    load_x(0, nc.scalar)
    nc.sync.dma_start(wkf[0][:], wkP_d[0])
    nc.sync.dma_start(wvt[0][:], wvP_d[0])
    nc.sync.dma_start(wqf[1][:], wqP_d[1])
    nc.sync.dma_start(wkf[1][:], wkP_d[1])
    nc.scalar.dma_start(mtri[:], mtri_d[:])
    nc.vector.memset(ones_bf[:], 1.0)
    load_x(1, nc.scalar)
    nc.sync.dma_start(wvt[1][:], wvP_d[1])
    nc.sync.dma_start(wot[:], woP_d[:])

    # ---- filler emitters: one call emits one projection group ----
    # all ps_st allocations are full [KB, 2*QC] slots so the pool slots are
    # uniform; projection groups just use a slice
    def qk_group(jq, wf, dst, fc):
        ps = ps_st.tile([KB, 2 * QC], F32, tag="st", name=f"pp{jq}_{fc}")[:, 0:QC]
        for hc in range(8):
            nc.tensor.matmul(
                ps[:],
                wf[fc][:, hc * KB : (hc + 1) * KB],
                xcs(jq, hc),
                start=(hc == 0),
                stop=(hc == 7),
            )
        nc.vector.tensor_copy(dst[fc][:, jq * QC : (jq + 1) * QC], ps[:])

    def v_group(jq, sub):
        rc = 4 * jq + sub
        psv = ps_st.tile([KB, 2 * QC], F32, tag="st", name=f"pv{rc}")[:, 0:F]
        for hc in range(8):
            nc.tensor.matmul(
                psv[:],
                xcs(jq, hc)[:, sub * KB : (sub + 1) * KB],
                wv[hc][:],
                start=(hc == 0),
                stop=(hc == 7),
            )
        v_heads = vts[rc][:].rearrange("p (h d) -> p h d", h=NHC)
        nc.vector.tensor_copy(
            v_heads[:, :, 0:HD], psv[:].rearrange("p (h d) -> p h d", h=NHC)
        )
        nc.vector.tensor_copy(
            v_heads[:, :, HD : 2 * HD],
            ones_bf[:, :].rearrange("p (h d) -> p h d", h=NHC),
        )

    def o_group(qb, act_copy=False):
        # pso lives in the st pool: during attention rounds the po pool's 4
        # banks are all held by the per-head accumulators, so allocating pso
        # there would deadlock the in-order PE queue.
        ost = ostage.tile([KB, 2 * QC], BF16, tag="ost", name=f"os{qb}")
        for oc in range(2):
            pso = ps_st.tile([KB, 2 * QC], F32, tag="st", name=f"pso{qb}_{oc}")
            pso = pso[:, 0:QC]
            for fc in range(2):
                nc.tensor.matmul(
                    pso[:],
                    ats[fc][:, qb * KB : (qb + 1) * KB],
                    wo[fc][:, oc * QC : (oc + 1) * QC],
                    start=(fc == 0),
                    stop=(fc == 1),
                )
            dst = ost[:, oc * QC : (oc + 1) * QC]
            if act_copy and oc == 0:
                nc.scalar.activation(dst, pso[:], COPY)
            else:
                nc.vector.tensor_copy(dst, pso[:])
        nc.sync.dma_start(out_d[qb * KB : (qb + 1) * KB, :], ost[:])

    # chunk 0's projections run before its attention
    for fc in range(2):
        qk_group(0, wqf, qts, fc)
        qk_group(0, wkf, kts, fc)
    for sub in range(4):
        v_group(0, sub)

    for jq in range(4):
        q0 = jq * QC
        nkb = 4 * jq + 4

        # filler work interleaved into this chunk's attention rounds, in
        # dependency order: this chunk's remaining V groups (needed by the
        # last 4 rounds only), next chunk's x prefetch + Q/K projections,
        # then the previous chunk's O projection (ats ready ~round 1).
        fillers = []
        if jq > 0:
            for sub in range(4):
                fillers.append(lambda j=jq, s=sub: v_group(j, s))
        if jq + 2 < 4:
            fillers.append(lambda j=jq + 2: load_x(j, nc.sync))
        if jq + 1 < 4:
            for fc in range(2):
                fillers.append(lambda j=jq + 1, f=fc: qk_group(j, wqf, qts, f))
                fillers.append(lambda j=jq + 1, f=fc: qk_group(j, wkf, kts, f))
        if jq > 0:
            for sub in range(4):
                fillers.append(lambda q=4 * (jq - 1) + sub: o_group(q))
        # spread fillers evenly over rounds 1..nkb-1 (round 0's PSUM slots
        # feed the next rounds' scores; a filler there funnels the PE queue
        # behind the first exp)
        # the last two filler groups are held back and emitted after the
        # final PV flush: they keep the PE warm through the normalize
        # chain at the chunk boundary
        reserved = fillers[-2:] if len(fillers) >= 2 else []
        fillers = fillers[: len(fillers) - len(reserved)]
        nf = len(fillers)
        sched = [min(nkb - 1, 1 + (i * (nkb - 1)) // nf) for i in range(nf)]
        fi = 0

        # ---- attention: 4 heads round-robin, PV lagging one k-block.
        # Scores for a head-pair land in one 2-bank PSUM supertile so a
        # single exp covers both heads (halves ACT per-instruction
        # overhead, the round pacer). ----
        po = [
            ps_po.tile([KB, QC], F32, tag="ot", name=f"po{jq}_{h}") for h in range(NHC)
        ]
        pend = []  # (pts, w0, ik) awaiting their PV pass; lag 2 rounds

        def flush_pv():
            pts_, w0_, ik_ = pend.pop(0)
            for h in range(NHC):
                pt2, base = pts_[h]
                nc.tensor.matmul(
                    po[h][:, w0_:QC],
                    vts[ik_][:, h * 2 * HD : (h + 1) * 2 * HD],
                    pt2[:, base + w0_ : base + QC],
                    start=(ik_ == 0),
                    stop=(ik_ == nkb - 1),
                )

        for ik in range(nkb):
            r = ik - 4 * jq
            w0 = max(r, 0) * KB  # fully-masked leading columns skipped
            pts = []
            for p in range(2):  # head pair (2p, 2p+1)
                st2 = ps_st.tile([KB, 2 * QC], F32, tag="st", name=f"st{jq}_{p}_{ik}")
                pt2 = pt_pool.tile([KB, 2 * QC], BF16, tag="pt", name=f"pt{jq}_{p}_{ik}")
                for hh in range(2):
                    h = 2 * p + hh
                    t, po_ = h // 2, (h % 2) * HD
                    nc.tensor.matmul(
                        st2[:, hh * QC + w0 : (hh + 1) * QC],
                        kts[t][po_ : po_ + HD, ik * KB : (ik + 1) * KB],
                        qts[t][po_ : po_ + HD, q0 + w0 : q0 + QC],
                        start=True,
                        stop=True,
                    )
                sv = st2[:].rearrange("p (h q) -> p h q", h=2)[:, :, w0:QC]
                pv_ = pt2[:].rearrange("p (h q) -> p h q", h=2)[:, :, w0:QC]
                nc.scalar.activation(pv_, sv, EXP, scale=SCALE)
                if r >= 0:  # causal triangle mask on the diagonal block
                    for hh in range(2):
                        tri = pt2[:, hh * QC + r * KB : hh * QC + (r + 1) * KB]
                        nc.vector.tensor_mul(tri, tri, mtri[:])
                pts.extend((pt2, hh * QC) for hh in range(2))
            pend.append((pts, w0, ik))
            if len(pend) > 3:
                flush_pv()
            while fi < nf and sched[fi] <= ik:
                fillers[fi]()
                fi += 1
        while fi < nf:
            fillers[fi]()
            fi += 1
        while pend:
            flush_pv()
        for fn in reserved:
            fn()
        # per-head normalize: stage the denominator (rows 64:128 of po, 64
        # replicated copies) to SBUF, approx-reciprocal, scale.  All on DVE
        # as per-head chains so the ACT queue stays free for the next
        # chunk's exps; the PV lag covers the latency.
        for h in range(NHC):
            t, po_ = h // 2, (h % 2) * HD
            den = recip_pool.tile([HD, QC], F32, tag="recip", name=f"dn{jq}_{h}")
            recip = recip_pool.tile([HD, QC], F32, tag="recip", name=f"rc{jq}_{h}")
            if jq == 3 and h % 2 == 0:
                # no more exps: ACT takes half the staging copies so the
                # DVE reciprocal chain starts sooner
                nc.scalar.activation(den[:], po[h][HD : 2 * HD, :], COPY)
            else:
                nc.vector.tensor_copy(den[:], po[h][HD : 2 * HD, :])
            with nc.allow_low_precision(reason="softmax denom"):
                nc.vector.reciprocal_approx_fast(recip[:], den[:])
                nc.vector.tensor_mul(
                    ats[t][po_ : po_ + HD, q0 : q0 + QC], po[h][0:HD, :], recip[:]
                )

    # last chunk's output projection (ACT is idle here, so it takes half
    # the PSUM->SBUF copies off the DVE critical path)
    for sub in range(4):
        o_group(12 + sub, act_copy=True)



def _build():
    if "nc" in _CACHE:
        return _CACHE["nc"]
    nc = bacc.Bacc(
        "TRN2", target_bir_lowering=False, debug=False, num_devices=N_CORES
    )
    with tile.TileContext(nc) as tc:
        with ExitStack() as ctx:
            tc._emit_ctx = ctx
            _emit(tc)
    nc.compile()
    _CACHE["nc"] = nc
    return nc


def _numpy_fallback(q, attention_mask, Wq, Wk, Wv, Wo):
    import math

    b, s, _ = q.shape
    causal = np.tril(np.ones((s, s), bool))
    valid = attention_mask != 0
    mask = causal[None] & valid[:, :, None] & valid[:, None, :]
    mask = mask[:, None]
    out = np.zeros((b, s, H), np.float32)
    for bi in range(b):
        x = q[bi]
        nh = x.shape[1] // HD
        qh = (x @ Wq.T).reshape(s, nh, HD).transpose(1, 0, 2)
        kh = (x @ Wk.T).reshape(s, nh, HD).transpose(1, 0, 2)
        vh = (x @ Wv.T).reshape(s, nh, HD).transpose(1, 0, 2)
        sc = np.einsum("hqd,hkd->hqk", qh, kh) / math.sqrt(HD)
        sc = np.where(mask[bi], sc, np.float32(-1e6))
        sc = sc - sc.max(-1, keepdims=True)
        e = np.exp(sc)
        p = e / e.sum(-1, keepdims=True)
        p = np.where(mask[bi], p, np.float32(0.0))
        o = np.einsum("hqk,hkd->hqd", p, vh).transpose(1, 0, 2).reshape(s, -1)
        out[bi] = o @ Wo.T
    return out


def _pack_x(xT):
    # xP[jq, half] = [128, 2048]: 4 hidden blocks of x^T side by side
    bf = ml_dtypes.bfloat16
    xP = np.empty((4, 2, KB, 4 * QC), dtype=bf)
    for jq in range(4):
        for half in range(2):
            for i in range(4):
                hc = half * 4 + i
                xP[jq, half, :, i * QC : (i + 1) * QC] = xT[
                    hc * KB : (hc + 1) * KB, jq * QC : (jq + 1) * QC
                ]
    return xP


def _pack_w(wT):
    # [n*128, W] -> [128, n*W]: hidden blocks side by side
    bf = ml_dtypes.bfloat16
    n = wT.shape[0] // KB
    out = np.empty((KB, n * wT.shape[1]), dtype=bf)
    for i in range(n):
        out[:, i * wT.shape[1] : (i + 1) * wT.shape[1]] = wT[i * KB : (i + 1) * KB, :]
    return out


def _pack_w_fc(wT):
    # [1024, 256] -> [2, 128, 1024]: per output-feature half fc, the 8
    # hidden blocks' [128, 128] stationaries side by side
    bf = ml_dtypes.bfloat16
    out = np.empty((2, KB, 8 * KB), dtype=bf)
    for fc in range(2):
        for hc in range(8):
            out[fc, :, hc * KB : (hc + 1) * KB] = wT[
                hc * KB : (hc + 1) * KB, fc * KB : (fc + 1) * KB
            ]
    return out


def _pack_w_half(wT):
    # [1024, 256] -> [2, 128, 1024]: two halves of 4 hidden blocks each
    bf = ml_dtypes.bfloat16
    out = np.empty((2, KB, 4 * F), dtype=bf)
    for i in range(2):
        for j in range(4):
            out[i, :, j * F : (j + 1) * F] = wT[(4 * i + j) * KB : (4 * i + j + 1) * KB, :]
    return out


def _run(q, attention_mask, Wq, Wk, Wv, Wo, trace=False, **trace_kwargs):
    q = np.ascontiguousarray(np.asarray(q, dtype=np.float32))
    Wq = np.asarray(Wq, dtype=np.float32)
    Wk = np.asarray(Wk, dtype=np.float32)
    Wv = np.asarray(Wv, dtype=np.float32)
    Wo = np.asarray(Wo, dtype=np.float32)
    am = np.asarray(attention_mask)
    if q.shape != (B, S, H) or not np.all(am != 0):
        return _numpy_fallback(q, am, Wq, Wk, Wv, Wo), None

    bf = ml_dtypes.bfloat16
    idx = np.arange(KB)
    mtri = (idx[:, None] <= idx[None, :]).astype(bf)

    in_maps = []
    for c in range(N_CORES):
        b, g = c // 4, c % 4
        fs = slice(F * g, F * (g + 1))
        in_maps.append(
            {
                "xP": _pack_x(q[b].T.astype(bf)),
                "wqP": _pack_w_fc(Wq[fs, :].T.astype(bf)),
                "wkP": _pack_w_fc(Wk[fs, :].T.astype(bf)),
                "wvP": _pack_w_half(Wv[fs, :].T.astype(bf)),
                "woP": _pack_w(Wo[:, fs].T.astype(bf)),
                "mtri": mtri,
            }
        )

    nc = _build()
    res = bass_utils.run_bass_kernel_spmd(
        nc, in_maps, core_ids=list(range(N_CORES)), trace=trace, **trace_kwargs
    )
    outs = [r["out"].astype(np.float32) for r in res.results]
    full = np.empty((B, S, H), np.float32)
    for b in range(B):
        full[b] = outs[4 * b] + outs[4 * b + 1] + outs[4 * b + 2] + outs[4 * b + 3]
    return full, res


def kernel(q, attention_mask, Wq, Wk, Wv, Wo):
    out, _ = _run(q, attention_mask, Wq, Wk, Wv, Wo)
    return out
